# revision 18
# baseline (speedup 1.0000x reference)
"""YOLOv1 loss (nn_LossModul_16277926052544) on 8 TRN2 NeuronCores.

Pure data parallel: batch 8192 -> 8 shards of 1024 (= 50176 grid cells/core,
128 partitions x F=392). Each core computes partial sums; host reduces.

Design notes (vs the 75us f32 baseline):
  * Box channels bf16, cls channels fp8-e3m4 in DRAM (numpy-validated
    rel err 7.0e-4 vs the 2e-2 gate). HBM reads: 11.3MB -> 3.9MB/core.
  * cls difference (pc - tc) is computed by the DMA engine: pcls is cast
    fp8->bf16 in flight (SWDGE), then -tcls lands on the same SBUF tile
    with accum_op=add. Removes a 20-channel DVE pass entirely.
  * dxy lands in-place over pb's xy channels so one contiguous 5ch
    copy + one predicated copy select the responsible box.
  * Doubled-overlap algebra: ln2 = min(max((pw+tw) - 2R|dxy|, 0), 2pw,
    2tw) = 2*overlap, so I'=4I, D'=4D; resp (I1'D2' > I2'D1') and
    iou (I'/D') are scale-invariant. This avoids SCALAR_TENSOR_TENSOR
    (which has no 2x bf16 uop and runs 1x) in favor of 2x TTs/4x TSs.
  * |dxy| runs on the Scalar engine as Abs(2R*dxy) via the activation's
    free affine scale; lambdas fold into per-cell masks so Square+accum
    reduces every loss term.
"""
import sys

for _p in ("/opt/trn_rl_repo",):
    if _p not in sys.path:
        sys.path.insert(0, _p)

import numpy as np
import ml_dtypes
from contextlib import ExitStack

import concourse.bass as bass  # noqa: F401  (registers engines)
from concourse import bacc, mybir
from concourse import bass_utils
import concourse.tile as tile

N_CORES = 8
BATCH = 8192
S = 7
P = 128
CELLS = (BATCH // N_CORES) * S * S            # 50176
FF = CELLS // P                               # 392
FH = FF // 2                                  # 196
R2 = 2.0 / S
EPS5 = 5e-6                                   # 5 * EPS (lambda folded)
SQRT5 = float(np.sqrt(5.0))
SQH = float(np.sqrt(0.5))

f32 = mybir.dt.float32
bf16 = mybir.dt.bfloat16
fp8 = mybir.dt.float8e3                       # e3m4
u16 = mybir.dt.uint16
Alu = mybir.AluOpType
Act = mybir.ActivationFunctionType

_CACHE = {}


def _build_body(tc, ctx, pb_d, tb_d, pc_d, tn_d, out_ap):
    nc = tc.nc
    pool = ctx.enter_context(tc.tile_pool(name="w", bufs=1))
    t = lambda shape, dt, tag: pool.tile(shape, dt, tag=tag, name=tag)

    eps5c = t([P, 1], f32, "eps5c")
    nc.gpsimd.memset(eps5c[:], EPS5)

    pb = t([P, 10, FF], bf16, "pb")       # [x1,y1,w1,h1,c1,x2,y2,w2,h2,c2]
    tb = t([P, 5, FF], bf16, "tb")        # [tx,ty,tw,th,tconf]
    dt_ = t([P, 27, FF], bf16, "dt")      # 0:20 cls diff, 20:27 sel slots
    stats = t([P, 6], f32, "stats")

    # ---- DMAs: box via HWDGE; cls via SWDGE with fp8->bf16 cast, then
    # -tcls accumulated on top (d = pc - tc materializes on arrival).
    # The SWDGE stream is chained behind pb via a 1-elem gpsimd read so
    # cls traffic doesn't round-robin against the box DMAs the geometry
    # pipeline is waiting on.
    nc.sync.dma_start(tb[:], tb_d)
    nc.sync.dma_start(pb[:], pb_d)
    # WAW chain: this 1-elem write into dt_ depends on pb, and the pcls
    # DMA (writing the same region) must follow it -> cls traffic starts
    # only after the box DMA the geometry pipeline is waiting on.
    nc.gpsimd.tensor_copy(dt_[:, 0, 0:1], pb[:, 0, 0:1])
    # CCE (DMA accumulate) caps at 2048 elems/descriptor: 5-channel chunks
    # (5*392 = 1960), each contiguous per partition. pcls is chunked the
    # same way so each accum only waits on its own chunk, not all of pcls.
    for c in range(0, 20, 5):
        nc.gpsimd.dma_start(dt_[:, c:c + 5], pc_d[:, c:c + 5])
    for c in range(0, 20, 5):
        nc.gpsimd.dma_start(dt_[:, c:c + 5], tn_d[:, c:c + 5],
                            accum_op=Alu.add)

    # ---- masks (lambdas folded in) ----
    M = t([P, 4, FF], bf16, "M")          # mo, sqrt5*mo, 5*mo, sqrt(.5)(1-mo)
    nc.vector.tensor_single_scalar(M[:, 0], tb[:, 4], 0.0, op=Alu.is_gt)
    nc.vector.tensor_scalar_mul(M[:, 1], M[:, 0], SQRT5)
    nc.vector.tensor_scalar_mul(M[:, 2], M[:, 0], 5.0)
    nc.vector.tensor_scalar(M[:, 3], M[:, 0], -SQH, SQH,
                            op0=Alu.mult, op1=Alu.add)
    mo = M[:, 0:1]                         # [P,1,FF] for broadcasts

    pbv = pb[:].rearrange("p (b c) f -> p b (c f)", b=2)   # [P,2,5*FF]
    p_xy = pbv[:, :, 0:2 * FF]
    p_wh = pbv[:, :, 2 * FF:4 * FF]
    tbf = tb[:].rearrange("p c f -> p (c f)")
    t_xy = tbf[:, 0:2 * FF].unsqueeze(1).broadcast_to([P, 2, 2 * FF])
    t_wh = tbf[:, 2 * FF:4 * FF].unsqueeze(1).broadcast_to([P, 2, 2 * FF])

    # tb-only hoistables
    twh2 = t([P, 2 * FF], bf16, "twh2")   # 2*twh
    nc.vector.tensor_scalar_mul(twh2[:], tbf[:, 2 * FF:4 * FF], 2.0)
    at2 = t([P, 1, FF], bf16, "at2")      # 4*tarea
    nc.vector.tensor_mul(at2[:, 0], twh2[:, 0:FF], twh2[:, FF:2 * FF])
    uv = t([P, 4, FF], bf16, "uv")        # [5mo*ws, 5mo*hs, 5mo*tw, 5mo*th]
    nc.vector.tensor_mul(uv[:, 2:4], tb[:, 2:4],
                         M[:, 2:3].broadcast_to([P, 2, FF]))

    # ---- geometry, both boxes as [P, 2, 2*FF] ----
    # dxy in-place over pb's xy channels (the 5ch select below needs it)
    nc.vector.tensor_sub(p_xy, p_xy, t_xy)
    nc.vector.tensor_mul(sel_noobj(dt_), pb[:, 4:10:5],
                         M[:, 3:4].broadcast_to([P, 2, FF]))
    absd2 = t([P, 2, 2 * FF], bf16, "absd2")   # 2R*|dxy|  (Scalar engine)
    nc.scalar.activation(absd2[:], p_xy, Act.Abs, scale=R2)
    sm2 = t([P, 2, 2 * FF], bf16, "sm2")       # pw+tw
    nc.vector.tensor_add(sm2[:], p_wh, t_wh)
    pwh2 = t([P, 2, 2 * FF], bf16, "pwh2")     # 2*pwh
    nc.vector.tensor_scalar_mul(pwh2[:], p_wh, 2.0)
    mm2 = t([P, 2, 2 * FF], bf16, "mm2")       # (pw+tw) - 2R|d|
    nc.vector.tensor_sub(mm2[:], sm2[:], absd2[:])
    nc.vector.tensor_single_scalar(mm2[:], mm2[:], 0.0, op=Alu.max)
    nc.vector.tensor_tensor(mm2[:], mm2[:], pwh2[:], op=Alu.min)
    ln2 = t([P, 2, 2 * FF], bf16, "ln2")       # 2*overlap
    nc.vector.tensor_tensor(
        ln2[:], mm2[:], twh2[:].unsqueeze(1).broadcast_to([P, 2, 2 * FF]),
        op=Alu.min)

    ID = t([P, 4, FF], bf16, "ID")        # [I1',I2',D1',D2'] (4x scaled)
    ln4 = ln2[:].rearrange("p b (a f) -> p b a f", a=2)
    nc.vector.tensor_mul(ID[:, 0:2], ln4[:, :, 0], ln4[:, :, 1])
    ap2 = t([P, 2, FF], bf16, "ap2")
    pwh2v = pwh2[:].rearrange("p b (c f) -> p b c f", c=2)
    nc.vector.tensor_mul(ap2[:], pwh2v[:, :, 0], pwh2v[:, :, 1])
    nc.vector.tensor_sub(ap2[:], ap2[:], ID[:, 0:2])
    nc.vector.tensor_add(ID[:, 2:4], ap2[:], at2[:].broadcast_to([P, 2, FF]))

    g = t([P, 2, FF], bf16, "g")
    nc.vector.tensor_mul(g[:, 0], ID[:, 0], ID[:, 3])
    nc.vector.tensor_mul(g[:, 1], ID[:, 1], ID[:, 2])
    resp = t([P, 1, FF], u16, "resp")     # 1 -> box1 responsible
    nc.vector.tensor_tensor(resp[:, 0], g[:, 0], g[:, 1], op=Alu.is_gt)

    # ---- selects: box2 copied, box1 predicated over it ----
    sel = dt_[:, 20:27]                   # [dxs,dys,ws,hs,cs,n1,n2]
    nc.scalar.copy(sel[:, 0:5], pb[:, 5:10])
    nc.vector.copy_predicated(sel[:, 0:5],
                              resp[:].broadcast_to([P, 5, FF]), pb[:, 0:5])
    idsel = t([P, 2, FF], bf16, "idsel")  # [Isel', Dsel']
    nc.vector.tensor_copy(idsel[:], ID[:, 1:4:2])
    nc.vector.copy_predicated(idsel[:], resp[:].broadcast_to([P, 2, FF]),
                              ID[:, 0:3:2])

    dful = t([P, 1, FF], f32, "dful")
    nc.vector.tensor_copy(dful[:], idsel[:, 1:2])
    rcp = t([P, 1, FF], f32, "rcp")
    nc.vector.reciprocal_approx_fast(rcp[:, 0], dful[:, 0])
    iou = t([P, 1, FF], f32, "iou")
    nc.vector.tensor_mul(iou[:, 0], idsel[:, 0], rcp[:, 0])
    nc.vector.scalar_tensor_tensor(sel[:, 4], iou[:, 0], -1.0, sel[:, 4],
                                   op0=Alu.mult, op1=Alu.add)
    nc.vector.tensor_mul(sel[:, 4], sel[:, 4], M[:, 0])

    # ---- masked slots ----
    nc.vector.tensor_mul(sel[:, 0:2], sel[:, 0:2],
                         M[:, 1:2].broadcast_to([P, 2, FF]))
    nc.vector.tensor_mul(uv[:, 0:2], sel[:, 2:4],
                         M[:, 2:3].broadcast_to([P, 2, FF]))
    w_ = t([P, 4, FF], bf16, "w")
    nc.scalar.activation(w_[:], uv[:], Act.Sqrt, bias=eps5c[:])
    nc.vector.tensor_sub(sel[:, 2:4], w_[:, 0:2], w_[:, 2:4])

    # ---- squares: sel + cls quarters, Square+accum on ACT ----
    nc.scalar.activation(sel[:], sel[:], Act.Square,
                         accum_out=stats[:, 0:1])
    FQ = FF // 4
    for h in range(4):
        fs = slice(h * FQ, (h + 1) * FQ)
        nc.vector.tensor_mul(dt_[:, 0:20, fs], dt_[:, 0:20, fs],
                             mo[:, :, fs].broadcast_to([P, 20, FQ]))
        nc.scalar.activation(dt_[:, 0:20, fs], dt_[:, 0:20, fs], Act.Square,
                             accum_out=stats[:, 1 + h:2 + h])

    nc.sync.dma_start(out_ap, stats[:])


def sel_noobj(dt_):
    return dt_[:, 25:27]


def _build():
    if "nc" in _CACHE:
        return _CACHE["nc"]
    nc = bacc.Bacc("TRN2", target_bir_lowering=False, debug=False)
    pb_d = nc.dram_tensor("pbox", [P, 10, FF], bf16, kind="ExternalInput")
    tb_d = nc.dram_tensor("tbox", [P, 5, FF], bf16, kind="ExternalInput")
    pc_d = nc.dram_tensor("pcls", [P, 20, FF], fp8, kind="ExternalInput")
    tn_d = nc.dram_tensor("tclsn", [P, 20, FF], fp8, kind="ExternalInput")
    out = nc.dram_tensor("out", [P, 6], f32, kind="ExternalOutput")
    with tile.TileContext(nc) as tc, ExitStack() as ctx:
        _build_body(tc, ctx, pb_d.ap(), tb_d.ap(), pc_d.ap(), tn_d.ap(),
                    out.ap())
    nc.compile()
    _CACHE["nc"] = nc
    return nc


def _shard(predicts, targets):
    p = np.ascontiguousarray(predicts, dtype=np.float32)
    tg = np.ascontiguousarray(targets, dtype=np.float32)
    n = BATCH // N_CORES
    maps = []
    for i in range(N_CORES):
        ps = p[i * n:(i + 1) * n].reshape(P, FF, 30).transpose(0, 2, 1)
        ts = tg[i * n:(i + 1) * n].reshape(P, FF, 30).transpose(0, 2, 1)
        maps.append({
            "pbox": np.ascontiguousarray(ps[:, 0:10]).astype(
                ml_dtypes.bfloat16),
            "tbox": np.ascontiguousarray(ts[:, 0:5]).astype(
                ml_dtypes.bfloat16),
            "pcls": np.ascontiguousarray(ps[:, 10:30]).astype(
                ml_dtypes.float8_e3m4),
            "tclsn": np.ascontiguousarray(-ts[:, 10:30]).astype(
                ml_dtypes.float8_e3m4),
        })
    return maps


def run(predicts, targets, trace=False, **trace_kwargs):
    nc = _build()
    in_maps = _shard(predicts, targets)
    res = bass_utils.run_bass_kernel_spmd(
        nc, in_maps, core_ids=list(range(N_CORES)), trace=trace,
        **trace_kwargs)
    partial = np.zeros((), dtype=np.float64)
    for r in res.results:
        partial += np.asarray(r["out"], dtype=np.float64).sum()
    return np.float32(partial), res


def kernel(predicts, targets):
    out, _ = run(predicts, targets, trace=False)
    return out


# revision 19
# speedup vs baseline: 1.0486x; 1.0486x over previous
"""YOLOv1 loss (nn_LossModul_16277926052544) on 8 TRN2 NeuronCores.

Pure data parallel: batch 8192 -> 8 shards of 1024 (= 50176 grid cells/core,
128 partitions x F=392). Each core computes partial sums; host reduces.

Design notes (vs the 75us f32 baseline):
  * Box channels bf16, cls channels fp8-e3m4 in DRAM (numpy-validated
    rel err 7.0e-4 vs the 2e-2 gate). HBM reads: 11.3MB -> 3.9MB/core.
  * cls difference (pc - tc) is computed by the DMA engine: pcls is cast
    fp8->bf16 in flight (SWDGE), then -tcls lands on the same SBUF tile
    with accum_op=add. Removes a 20-channel DVE pass entirely.
  * dxy lands in-place over pb's xy channels so one contiguous 5ch
    copy + one predicated copy select the responsible box.
  * Doubled-overlap algebra: ln2 = min(max((pw+tw) - 2R|dxy|, 0), 2pw,
    2tw) = 2*overlap, so I'=4I, D'=4D; resp (I1'D2' > I2'D1') and
    iou (I'/D') are scale-invariant. This avoids SCALAR_TENSOR_TENSOR
    (which has no 2x bf16 uop and runs 1x) in favor of 2x TTs/4x TSs.
  * |dxy| runs on the Scalar engine as Abs(2R*dxy) via the activation's
    free affine scale; lambdas fold into per-cell masks so Square+accum
    reduces every loss term.
"""
import sys

for _p in ("/opt/trn_rl_repo",):
    if _p not in sys.path:
        sys.path.insert(0, _p)

import numpy as np
import ml_dtypes
from contextlib import ExitStack

import concourse.bass as bass  # noqa: F401  (registers engines)
from concourse import bacc, mybir
from concourse import bass_utils
import concourse.tile as tile

N_CORES = 8
BATCH = 8192
S = 7
P = 128
CELLS = (BATCH // N_CORES) * S * S            # 50176
FF = CELLS // P                               # 392
FH = FF // 2                                  # 196
R2 = 2.0 / S
EPS5 = 5e-6                                   # 5 * EPS (lambda folded)
SQRT5 = float(np.sqrt(5.0))
SQH = float(np.sqrt(0.5))

f32 = mybir.dt.float32
bf16 = mybir.dt.bfloat16
fp8 = mybir.dt.float8e3                       # e3m4
u16 = mybir.dt.uint16
Alu = mybir.AluOpType
Act = mybir.ActivationFunctionType

_CACHE = {}


def _build_body(tc, ctx, pb_d, tb_d, pc_d, tn_d, out_ap):
    nc = tc.nc
    pool = ctx.enter_context(tc.tile_pool(name="w", bufs=1))
    t = lambda shape, dt, tag: pool.tile(shape, dt, tag=tag, name=tag)

    eps5c = t([P, 1], f32, "eps5c")
    nc.gpsimd.memset(eps5c[:], EPS5)

    pb = t([P, 10, FF], bf16, "pb")       # [x1,y1,w1,h1,c1,x2,y2,w2,h2,c2]
    tb = t([P, 5, FF], bf16, "tb")        # [tx,ty,tw,th,tconf]
    dt_ = t([P, 27, FF], bf16, "dt")      # 0:20 cls diff, 20:27 sel slots
    stats = t([P, 6], f32, "stats")

    # ---- DMAs: box via HWDGE; cls via SWDGE with fp8->bf16 cast, then
    # -tcls accumulated on top (d = pc - tc materializes on arrival).
    # The SWDGE stream is chained behind pb via a 1-elem gpsimd read so
    # cls traffic doesn't round-robin against the box DMAs the geometry
    # pipeline is waiting on.
    nc.sync.dma_start(tb[:], tb_d)
    nc.sync.dma_start(pb[:], pb_d)
    # WAW chain: this 1-elem write into dt_ depends on pb, and the pcls
    # DMA (writing the same region) must follow it -> cls traffic starts
    # only after the box DMA the geometry pipeline is waiting on.
    nc.gpsimd.tensor_copy(dt_[:, 0, 0:1], pb[:, 0, 0:1])
    nc.gpsimd.dma_start(dt_[:, 0:20], pc_d)
    # CCE (DMA accumulate) caps at 2048 elems/descriptor: chunk the accum
    # into 5-channel groups (5*392 = 1960), each contiguous per partition.
    for c in range(0, 20, 5):
        nc.gpsimd.dma_start(dt_[:, c:c + 5], tn_d[:, c:c + 5],
                            accum_op=Alu.add)

    # ---- masks (lambdas folded in) ----
    M = t([P, 4, FF], bf16, "M")          # mo, sqrt5*mo, 5*mo, sqrt(.5)(1-mo)
    nc.vector.tensor_single_scalar(M[:, 0], tb[:, 4], 0.0, op=Alu.is_gt)
    nc.vector.tensor_scalar_mul(M[:, 1], M[:, 0], SQRT5)
    nc.vector.tensor_scalar_mul(M[:, 2], M[:, 0], 5.0)
    nc.vector.tensor_scalar(M[:, 3], M[:, 0], -SQH, SQH,
                            op0=Alu.mult, op1=Alu.add)
    mo = M[:, 0:1]                         # [P,1,FF] for broadcasts

    pbv = pb[:].rearrange("p (b c) f -> p b (c f)", b=2)   # [P,2,5*FF]
    p_xy = pbv[:, :, 0:2 * FF]
    p_wh = pbv[:, :, 2 * FF:4 * FF]
    tbf = tb[:].rearrange("p c f -> p (c f)")
    t_xy = tbf[:, 0:2 * FF].unsqueeze(1).broadcast_to([P, 2, 2 * FF])
    t_wh = tbf[:, 2 * FF:4 * FF].unsqueeze(1).broadcast_to([P, 2, 2 * FF])

    # tb-only hoistables
    twh2 = t([P, 2 * FF], bf16, "twh2")   # 2*twh
    nc.vector.tensor_scalar_mul(twh2[:], tbf[:, 2 * FF:4 * FF], 2.0)
    at2 = t([P, 1, FF], bf16, "at2")      # 4*tarea
    nc.vector.tensor_mul(at2[:, 0], twh2[:, 0:FF], twh2[:, FF:2 * FF])
    uv = t([P, 4, FF], bf16, "uv")        # [5mo*ws, 5mo*hs, 5mo*tw, 5mo*th]
    nc.vector.tensor_mul(uv[:, 2:4], tb[:, 2:4],
                         M[:, 2:3].broadcast_to([P, 2, FF]))

    # ---- geometry, both boxes as [P, 2, 2*FF] ----
    # dxy in-place over pb's xy channels (the 5ch select below needs it)
    nc.vector.tensor_sub(p_xy, p_xy, t_xy)
    nc.vector.tensor_mul(sel_noobj(dt_), pb[:, 4:10:5],
                         M[:, 3:4].broadcast_to([P, 2, FF]))
    absd2 = t([P, 2, 2 * FF], bf16, "absd2")   # 2R*|dxy|  (Scalar engine)
    nc.scalar.activation(absd2[:], p_xy, Act.Abs, scale=R2)
    sm2 = t([P, 2, 2 * FF], bf16, "sm2")       # pw+tw
    nc.vector.tensor_add(sm2[:], p_wh, t_wh)
    pwh2 = t([P, 2, 2 * FF], bf16, "pwh2")     # 2*pwh
    nc.vector.tensor_scalar_mul(pwh2[:], p_wh, 2.0)
    mm2 = t([P, 2, 2 * FF], bf16, "mm2")       # (pw+tw) - 2R|d|
    nc.vector.tensor_sub(mm2[:], sm2[:], absd2[:])
    nc.vector.tensor_single_scalar(mm2[:], mm2[:], 0.0, op=Alu.max)
    nc.vector.tensor_tensor(mm2[:], mm2[:], pwh2[:], op=Alu.min)
    ln2 = t([P, 2, 2 * FF], bf16, "ln2")       # 2*overlap
    nc.vector.tensor_tensor(
        ln2[:], mm2[:], twh2[:].unsqueeze(1).broadcast_to([P, 2, 2 * FF]),
        op=Alu.min)

    ID = t([P, 4, FF], bf16, "ID")        # [I1',I2',D1',D2'] (4x scaled)
    ln4 = ln2[:].rearrange("p b (a f) -> p b a f", a=2)
    nc.vector.tensor_mul(ID[:, 0:2], ln4[:, :, 0], ln4[:, :, 1])
    ap2 = t([P, 2, FF], bf16, "ap2")
    pwh2v = pwh2[:].rearrange("p b (c f) -> p b c f", c=2)
    nc.vector.tensor_mul(ap2[:], pwh2v[:, :, 0], pwh2v[:, :, 1])
    nc.vector.tensor_sub(ap2[:], ap2[:], ID[:, 0:2])
    nc.vector.tensor_add(ID[:, 2:4], ap2[:], at2[:].broadcast_to([P, 2, FF]))

    g = t([P, 2, FF], bf16, "g")
    nc.vector.tensor_mul(g[:, 0], ID[:, 0], ID[:, 3])
    nc.vector.tensor_mul(g[:, 1], ID[:, 1], ID[:, 2])
    resp = t([P, 1, FF], u16, "resp")     # 1 -> box1 responsible
    nc.vector.tensor_tensor(resp[:, 0], g[:, 0], g[:, 1], op=Alu.is_gt)

    # ---- selects: box2 copied, box1 predicated over it ----
    sel = dt_[:, 20:27]                   # [dxs,dys,ws,hs,cs,n1,n2]
    nc.scalar.copy(sel[:, 0:5], pb[:, 5:10])
    nc.vector.copy_predicated(sel[:, 0:5],
                              resp[:].broadcast_to([P, 5, FF]), pb[:, 0:5])
    idsel = t([P, 2, FF], bf16, "idsel")  # [Isel', Dsel']
    nc.vector.tensor_copy(idsel[:], ID[:, 1:4:2])
    nc.vector.copy_predicated(idsel[:], resp[:].broadcast_to([P, 2, FF]),
                              ID[:, 0:3:2])

    dful = t([P, 1, FF], f32, "dful")
    nc.vector.tensor_copy(dful[:], idsel[:, 1:2])
    rcp = t([P, 1, FF], f32, "rcp")
    nc.vector.reciprocal_approx_fast(rcp[:, 0], dful[:, 0])
    iou = t([P, 1, FF], f32, "iou")
    nc.vector.tensor_mul(iou[:, 0], idsel[:, 0], rcp[:, 0])
    nc.vector.scalar_tensor_tensor(sel[:, 4], iou[:, 0], -1.0, sel[:, 4],
                                   op0=Alu.mult, op1=Alu.add)
    nc.vector.tensor_mul(sel[:, 4], sel[:, 4], M[:, 0])

    # ---- masked slots ----
    nc.vector.tensor_mul(sel[:, 0:2], sel[:, 0:2],
                         M[:, 1:2].broadcast_to([P, 2, FF]))
    nc.vector.tensor_mul(uv[:, 0:2], sel[:, 2:4],
                         M[:, 2:3].broadcast_to([P, 2, FF]))
    w_ = t([P, 4, FF], bf16, "w")
    nc.scalar.activation(w_[:], uv[:], Act.Sqrt, bias=eps5c[:])
    nc.vector.tensor_sub(sel[:, 2:4], w_[:, 0:2], w_[:, 2:4])

    # ---- squares: sel + cls halves on ACT ----
    nc.scalar.activation(sel[:], sel[:], Act.Square,
                         accum_out=stats[:, 0:1])
    FQ = FF // 4
    for h in range(4):
        fs = slice(h * FQ, (h + 1) * FQ)
        nc.vector.tensor_mul(dt_[:, 0:20, fs], dt_[:, 0:20, fs],
                             mo[:, :, fs].broadcast_to([P, 20, FQ]))
        nc.scalar.activation(dt_[:, 0:20, fs], dt_[:, 0:20, fs], Act.Square,
                             accum_out=stats[:, 1 + h:2 + h])

    nc.sync.dma_start(out_ap, stats[:])


def sel_noobj(dt_):
    return dt_[:, 25:27]


def _build():
    if "nc" in _CACHE:
        return _CACHE["nc"]
    nc = bacc.Bacc("TRN2", target_bir_lowering=False, debug=False)
    pb_d = nc.dram_tensor("pbox", [P, 10, FF], bf16, kind="ExternalInput")
    tb_d = nc.dram_tensor("tbox", [P, 5, FF], bf16, kind="ExternalInput")
    pc_d = nc.dram_tensor("pcls", [P, 20, FF], fp8, kind="ExternalInput")
    tn_d = nc.dram_tensor("tclsn", [P, 20, FF], fp8, kind="ExternalInput")
    out = nc.dram_tensor("out", [P, 6], f32, kind="ExternalOutput")
    with tile.TileContext(nc) as tc, ExitStack() as ctx:
        _build_body(tc, ctx, pb_d.ap(), tb_d.ap(), pc_d.ap(), tn_d.ap(),
                    out.ap())
    nc.compile()
    _CACHE["nc"] = nc
    return nc


def _shard(predicts, targets):
    p = np.ascontiguousarray(predicts, dtype=np.float32)
    tg = np.ascontiguousarray(targets, dtype=np.float32)
    n = BATCH // N_CORES
    maps = []
    for i in range(N_CORES):
        ps = p[i * n:(i + 1) * n].reshape(P, FF, 30).transpose(0, 2, 1)
        ts = tg[i * n:(i + 1) * n].reshape(P, FF, 30).transpose(0, 2, 1)
        maps.append({
            "pbox": np.ascontiguousarray(ps[:, 0:10]).astype(
                ml_dtypes.bfloat16),
            "tbox": np.ascontiguousarray(ts[:, 0:5]).astype(
                ml_dtypes.bfloat16),
            "pcls": np.ascontiguousarray(ps[:, 10:30]).astype(
                ml_dtypes.float8_e3m4),
            "tclsn": np.ascontiguousarray(-ts[:, 10:30]).astype(
                ml_dtypes.float8_e3m4),
        })
    return maps


def run(predicts, targets, trace=False, **trace_kwargs):
    nc = _build()
    in_maps = _shard(predicts, targets)
    res = bass_utils.run_bass_kernel_spmd(
        nc, in_maps, core_ids=list(range(N_CORES)), trace=trace,
        **trace_kwargs)
    partial = np.zeros((), dtype=np.float64)
    for r in res.results:
        partial += np.asarray(r["out"], dtype=np.float64).sum()
    return np.float32(partial), res


def kernel(predicts, targets):
    out, _ = run(predicts, targets, trace=False)
    return out


# revision 21
# speedup vs baseline: 1.1586x; 1.1049x over previous
"""YOLOv1 loss (nn_LossModul_16277926052544) on 8 TRN2 NeuronCores.

Pure data parallel: batch 8192 -> 8 shards of 1024 (= 50176 grid cells/core,
128 partitions x F=392). Each core computes partial sums; host reduces.

Design notes (vs the 75us f32 baseline):
  * Box channels bf16, cls channels fp8-e3m4 in DRAM (numpy-validated
    rel err 7.0e-4 vs the 2e-2 gate). HBM reads: 11.3MB -> 3.9MB/core.
  * cls difference (pc - tc) is computed by the DMA engine: pcls is cast
    fp8->bf16 in flight (SWDGE), then -tcls lands on the same SBUF tile
    with accum_op=add. Removes a 20-channel DVE pass entirely.
  * dxy lands in-place over pb's xy channels so one contiguous 5ch
    copy + one predicated copy select the responsible box.
  * Doubled-overlap algebra: ln2 = min(max((pw+tw) - 2R|dxy|, 0), 2pw,
    2tw) = 2*overlap, so I'=4I, D'=4D; resp (I1'D2' > I2'D1') and
    iou (I'/D') are scale-invariant. This avoids SCALAR_TENSOR_TENSOR
    (which has no 2x bf16 uop and runs 1x) in favor of 2x TTs/4x TSs.
  * |dxy| runs on the Scalar engine as Abs(2R*dxy) via the activation's
    free affine scale; lambdas fold into per-cell masks so Square+accum
    reduces every loss term.
"""
import sys

for _p in ("/opt/trn_rl_repo",):
    if _p not in sys.path:
        sys.path.insert(0, _p)

import numpy as np
import ml_dtypes
from contextlib import ExitStack

import concourse.bass as bass  # noqa: F401  (registers engines)
from concourse import bacc, mybir
from concourse import bass_utils
import concourse.tile as tile

N_CORES = 8
BATCH = 8192
S = 7
P = 128
CELLS = (BATCH // N_CORES) * S * S            # 50176
FF = CELLS // P                               # 392
FH = FF // 2                                  # 196
R2 = 2.0 / S
EPS5 = 5e-6                                   # 5 * EPS (lambda folded)
SQRT5 = float(np.sqrt(5.0))
SQH = float(np.sqrt(0.5))

f32 = mybir.dt.float32
bf16 = mybir.dt.bfloat16
fp8 = mybir.dt.float8e3                       # e3m4
u16 = mybir.dt.uint16
Alu = mybir.AluOpType
Act = mybir.ActivationFunctionType

_CACHE = {}


def _build_body(tc, ctx, pb_d, tb_d, pc_d, tn_d, out_ap):
    nc = tc.nc
    pool = ctx.enter_context(tc.tile_pool(name="w", bufs=1))
    t = lambda shape, dt, tag: pool.tile(shape, dt, tag=tag, name=tag)

    eps5c = t([P, 1], f32, "eps5c")
    nc.gpsimd.memset(eps5c[:], EPS5)

    pb = t([P, 10, FF], bf16, "pb")       # [x1,y1,w1,h1,c1,x2,y2,w2,h2,c2]
    tb = t([P, 5, FF], bf16, "tb")        # [tx,ty,tw,th,tconf]
    dt_ = t([P, 27, FF], bf16, "dt")      # 0:20 cls diff, 20:27 sel slots
    stats = t([P, 6], f32, "stats")

    # ---- DMAs: box via HWDGE; cls via SWDGE with fp8->bf16 cast, then
    # -tcls accumulated on top (d = pc - tc materializes on arrival).
    # The SWDGE stream is chained behind pb via a 1-elem gpsimd read so
    # cls traffic doesn't round-robin against the box DMAs the geometry
    # pipeline is waiting on.
    nc.sync.dma_start(tb[:], tb_d)
    nc.sync.dma_start(pb[:], pb_d)
    # WAW chain: this 1-elem write into dt_ depends on tb, and the pcls
    # DMA (writing the same region) must follow it -> cls traffic yields
    # the first bandwidth window to the box DMAs the geometry pipeline
    # is waiting on, but still starts early enough that the accumulate
    # chain lands before the Vector engine drains its geometry work.
    nc.gpsimd.tensor_copy(dt_[:, 0, 0:1], tb[:, 0, 0:1])
    nc.gpsimd.dma_start(dt_[:, 0:20], pc_d)
    # CCE (DMA accumulate) caps at 2048 elems/descriptor: chunk the accum
    # into 5-channel groups (5*392 = 1960), each contiguous per partition.
    for c in range(0, 20, 5):
        nc.gpsimd.dma_start(dt_[:, c:c + 5], tn_d[:, c:c + 5],
                            accum_op=Alu.add)

    # ---- masks (lambdas folded in) ----
    M = t([P, 4, FF], bf16, "M")          # mo, sqrt5*mo, 5*mo, sqrt(.5)(1-mo)
    nc.vector.tensor_single_scalar(M[:, 0], tb[:, 4], 0.0, op=Alu.is_gt)
    nc.vector.tensor_scalar_mul(M[:, 1], M[:, 0], SQRT5)
    nc.vector.tensor_scalar_mul(M[:, 2], M[:, 0], 5.0)
    nc.vector.tensor_scalar(M[:, 3], M[:, 0], -SQH, SQH,
                            op0=Alu.mult, op1=Alu.add)
    mo = M[:, 0:1]                         # [P,1,FF] for broadcasts

    pbv = pb[:].rearrange("p (b c) f -> p b (c f)", b=2)   # [P,2,5*FF]
    p_xy = pbv[:, :, 0:2 * FF]
    p_wh = pbv[:, :, 2 * FF:4 * FF]
    tbf = tb[:].rearrange("p c f -> p (c f)")
    t_xy = tbf[:, 0:2 * FF].unsqueeze(1).broadcast_to([P, 2, 2 * FF])
    t_wh = tbf[:, 2 * FF:4 * FF].unsqueeze(1).broadcast_to([P, 2, 2 * FF])

    # tb-only hoistables
    twh2 = t([P, 2 * FF], bf16, "twh2")   # 2*twh
    nc.vector.tensor_scalar_mul(twh2[:], tbf[:, 2 * FF:4 * FF], 2.0)
    at2 = t([P, 1, FF], bf16, "at2")      # 4*tarea
    nc.vector.tensor_mul(at2[:, 0], twh2[:, 0:FF], twh2[:, FF:2 * FF])
    uv = t([P, 4, FF], bf16, "uv")        # [5mo*ws, 5mo*hs, 5mo*tw, 5mo*th]
    nc.vector.tensor_mul(uv[:, 2:4], tb[:, 2:4],
                         M[:, 2:3].broadcast_to([P, 2, FF]))

    # ---- geometry, both boxes as [P, 2, 2*FF] ----
    # dxy in-place over pb's xy channels (the 5ch select below needs it)
    nc.vector.tensor_sub(p_xy, p_xy, t_xy)
    nc.vector.tensor_mul(sel_noobj(dt_), pb[:, 4:10:5],
                         M[:, 3:4].broadcast_to([P, 2, FF]))
    absd2 = t([P, 2, 2 * FF], bf16, "absd2")   # 2R*|dxy|  (Scalar engine)
    nc.scalar.activation(absd2[:], p_xy, Act.Abs, scale=R2)
    sm2 = t([P, 2, 2 * FF], bf16, "sm2")       # pw+tw
    nc.vector.tensor_add(sm2[:], p_wh, t_wh)
    pwh2 = t([P, 2, 2 * FF], bf16, "pwh2")     # 2*pwh
    nc.vector.tensor_scalar_mul(pwh2[:], p_wh, 2.0)
    mm2 = t([P, 2, 2 * FF], bf16, "mm2")       # (pw+tw) - 2R|d|
    nc.vector.tensor_sub(mm2[:], sm2[:], absd2[:])
    nc.vector.tensor_single_scalar(mm2[:], mm2[:], 0.0, op=Alu.max)
    nc.vector.tensor_tensor(mm2[:], mm2[:], pwh2[:], op=Alu.min)
    ln2 = t([P, 2, 2 * FF], bf16, "ln2")       # 2*overlap
    nc.vector.tensor_tensor(
        ln2[:], mm2[:], twh2[:].unsqueeze(1).broadcast_to([P, 2, 2 * FF]),
        op=Alu.min)

    ID = t([P, 4, FF], bf16, "ID")        # [I1',I2',D1',D2'] (4x scaled)
    ln4 = ln2[:].rearrange("p b (a f) -> p b a f", a=2)
    nc.vector.tensor_mul(ID[:, 0:2], ln4[:, :, 0], ln4[:, :, 1])
    ap2 = t([P, 2, FF], bf16, "ap2")
    pwh2v = pwh2[:].rearrange("p b (c f) -> p b c f", c=2)
    nc.vector.tensor_mul(ap2[:], pwh2v[:, :, 0], pwh2v[:, :, 1])
    nc.vector.tensor_sub(ap2[:], ap2[:], ID[:, 0:2])
    nc.vector.tensor_add(ID[:, 2:4], ap2[:], at2[:].broadcast_to([P, 2, FF]))

    g = t([P, 2, FF], bf16, "g")
    nc.vector.tensor_mul(g[:, 0], ID[:, 0], ID[:, 3])
    nc.vector.tensor_mul(g[:, 1], ID[:, 1], ID[:, 2])
    resp = t([P, 1, FF], u16, "resp")     # 1 -> box1 responsible
    nc.vector.tensor_tensor(resp[:, 0], g[:, 0], g[:, 1], op=Alu.is_gt)

    # ---- selects: box2 copied, box1 predicated over it ----
    sel = dt_[:, 20:27]                   # [dxs,dys,ws,hs,cs,n1,n2]
    nc.scalar.copy(sel[:, 0:5], pb[:, 5:10])
    nc.vector.copy_predicated(sel[:, 0:5],
                              resp[:].broadcast_to([P, 5, FF]), pb[:, 0:5])
    idsel = t([P, 2, FF], bf16, "idsel")  # [Isel', Dsel']
    nc.vector.tensor_copy(idsel[:], ID[:, 1:4:2])
    nc.vector.copy_predicated(idsel[:], resp[:].broadcast_to([P, 2, FF]),
                              ID[:, 0:3:2])

    dful = t([P, 1, FF], f32, "dful")
    nc.vector.tensor_copy(dful[:], idsel[:, 1:2])
    rcp = t([P, 1, FF], f32, "rcp")
    nc.vector.reciprocal_approx_fast(rcp[:, 0], dful[:, 0])
    iou = t([P, 1, FF], f32, "iou")
    nc.vector.tensor_mul(iou[:, 0], idsel[:, 0], rcp[:, 0])
    nc.vector.scalar_tensor_tensor(sel[:, 4], iou[:, 0], -1.0, sel[:, 4],
                                   op0=Alu.mult, op1=Alu.add)
    nc.vector.tensor_mul(sel[:, 4], sel[:, 4], M[:, 0])

    # ---- masked slots ----
    nc.vector.tensor_mul(sel[:, 0:2], sel[:, 0:2],
                         M[:, 1:2].broadcast_to([P, 2, FF]))
    nc.vector.tensor_mul(uv[:, 0:2], sel[:, 2:4],
                         M[:, 2:3].broadcast_to([P, 2, FF]))
    w_ = t([P, 4, FF], bf16, "w")
    nc.scalar.activation(w_[:], uv[:], Act.Sqrt, bias=eps5c[:])
    nc.vector.tensor_sub(sel[:, 2:4], w_[:, 0:2], w_[:, 2:4])

    # ---- squares: sel + cls on ACT. The cls mask/square chunks are
    # sliced by the SAME 5-channel groups as the accumulate DMAs, so
    # each square fires as soon as its own accum chunk lands instead of
    # waiting for all four.
    nc.scalar.activation(sel[:], sel[:], Act.Square,
                         accum_out=stats[:, 0:1])
    for h, c in enumerate(range(0, 20, 5)):
        nc.vector.tensor_mul(dt_[:, c:c + 5], dt_[:, c:c + 5],
                             mo[:].broadcast_to([P, 5, FF]))
        nc.scalar.activation(dt_[:, c:c + 5], dt_[:, c:c + 5], Act.Square,
                             accum_out=stats[:, 1 + h:2 + h])

    nc.sync.dma_start(out_ap, stats[:])


def sel_noobj(dt_):
    return dt_[:, 25:27]


def _build():
    if "nc" in _CACHE:
        return _CACHE["nc"]
    nc = bacc.Bacc("TRN2", target_bir_lowering=False, debug=False)
    pb_d = nc.dram_tensor("pbox", [P, 10, FF], bf16, kind="ExternalInput")
    tb_d = nc.dram_tensor("tbox", [P, 5, FF], bf16, kind="ExternalInput")
    pc_d = nc.dram_tensor("pcls", [P, 20, FF], fp8, kind="ExternalInput")
    tn_d = nc.dram_tensor("tclsn", [P, 20, FF], fp8, kind="ExternalInput")
    out = nc.dram_tensor("out", [P, 6], f32, kind="ExternalOutput")
    with tile.TileContext(nc) as tc, ExitStack() as ctx:
        _build_body(tc, ctx, pb_d.ap(), tb_d.ap(), pc_d.ap(), tn_d.ap(),
                    out.ap())
    nc.compile()
    _CACHE["nc"] = nc
    return nc


def _shard(predicts, targets):
    p = np.ascontiguousarray(predicts, dtype=np.float32)
    tg = np.ascontiguousarray(targets, dtype=np.float32)
    n = BATCH // N_CORES
    maps = []
    for i in range(N_CORES):
        ps = p[i * n:(i + 1) * n].reshape(P, FF, 30).transpose(0, 2, 1)
        ts = tg[i * n:(i + 1) * n].reshape(P, FF, 30).transpose(0, 2, 1)
        maps.append({
            "pbox": np.ascontiguousarray(ps[:, 0:10]).astype(
                ml_dtypes.bfloat16),
            "tbox": np.ascontiguousarray(ts[:, 0:5]).astype(
                ml_dtypes.bfloat16),
            "pcls": np.ascontiguousarray(ps[:, 10:30]).astype(
                ml_dtypes.float8_e3m4),
            "tclsn": np.ascontiguousarray(-ts[:, 10:30]).astype(
                ml_dtypes.float8_e3m4),
        })
    return maps


def run(predicts, targets, trace=False, **trace_kwargs):
    nc = _build()
    in_maps = _shard(predicts, targets)
    res = bass_utils.run_bass_kernel_spmd(
        nc, in_maps, core_ids=list(range(N_CORES)), trace=trace,
        **trace_kwargs)
    partial = np.zeros((), dtype=np.float64)
    for r in res.results:
        partial += np.asarray(r["out"], dtype=np.float64).sum()
    return np.float32(partial), res


def kernel(predicts, targets):
    out, _ = run(predicts, targets, trace=False)
    return out
